# revision 23
# baseline (speedup 1.0000x reference)
"""AR(64) trajectory sampler on 8 trn2 NeuronCores.

reference: means[t] = AR(64) recurrence (deterministic, shared across batch),
           out[b, t] = means[t] + 0.3 * noise[b, t],  noise [256, 65536] f32.

Strategy (per sharding hint): replicate params/bias-derived small tensors,
shard the noise batch dim across 8 cores (32 rows each). The length-T scan
is parallelized via the companion-matrix block formulation:
    means[512*p + q] = (sigma_p . A'[q]) + c'[q]*b ,  sigma_{p+1} = M' sigma_p + d'
so the device materializes means as one [65,128]^T @ [65,512] matmul and
then streams out = 0.3*noise + means (memory-bound part).

Host work is limited to deriving the O(p^2)-sized block matrices from the
64-element params vector (and, in HOST_MEANS mode, the 128-step state scan).
"""

import os
import sys

import numpy as np

for _p in ("/root/.axon_site/_ro/trn_rl_repo", "/opt/trn_rl_repo"):
    if _p not in sys.path and os.path.isdir(_p):
        sys.path.append(_p)

from concourse import bacc, bass, tile
from concourse import mybir
from concourse.bass_utils import run_bass_kernel_spmd

F32 = mybir.dt.float32

BATCH = 256
MAX_T = 65536
P_ORDER = 64
STD = 0.3
N_CORES = 8
ROWS = BATCH // N_CORES          # 32 noise rows per core
L = 512                          # block length; T partitions = MAX_T // L = 128
NP_T = MAX_T // L                # 128 partitions of the means tile
# chunk sizes in rows (512KB/row-pair): small first chunks let stores start
# early (mixed read+write sustains ~410GB/s vs ~385 single-direction); small
# last chunk shrinks the serial load->compute->store tail.
CHUNKS = [2, 2, 4, 4, 4, 4, 4, 4, 2, 2]
assert sum(CHUNKS) == ROWS


def _derive_blocks(params: np.ndarray, bias: np.ndarray):
    """Block-companion expansion of the AR(64) recurrence, in float64.

    Returns (A, cb, Mp, dp):
      A  [L, 64] : row q maps state sigma -> means offset q within a block
      cb [L]     : additive term (bias folded in)
      Mp [64,64] : state advance over one block of L steps
      dp [64]    : additive state term over one block
    with state sigma = [m_{t-1}, ..., m_{t-64}] (most-recent-first).
    """
    a = params.astype(np.float64)
    b = float(bias[0])
    p = P_ORDER
    U = np.zeros((L, p), np.float64)
    e = np.zeros(L, np.float64)
    for i in range(L):
        u = np.zeros(p, np.float64)
        if i < p:
            u[: p - i] += a[i:]
        kmax = min(i, p)
        if kmax:
            u += a[:kmax] @ U[i - kmax : i][::-1]
            e[i] = 1.0 + a[:kmax] @ e[i - kmax : i][::-1]
        else:
            e[i] = 1.0
        U[i] = u
    A = U
    cb = e * b
    Mp = A[L - p :][::-1].copy()
    dp = cb[L - p :][::-1].copy()
    return A, cb, Mp, dp


def _host_means_tile(params: np.ndarray, bias: np.ndarray) -> np.ndarray:
    """means as [128, 512] f32: row p = means[512p : 512(p+1)]."""
    A, cb, Mp, dp = _derive_blocks(params, bias)
    sig = np.zeros((NP_T, P_ORDER), np.float64)
    for j in range(NP_T - 1):
        sig[j + 1] = Mp @ sig[j] + dp
    means = sig @ A.T + cb[None, :]
    return means.astype(np.float32)


N_SEED = 16  # prefix states computed on host; device doubles 16 -> 128
N_LEVELS = 3  # 16 -> 32 -> 64 -> 128
SMALL_COLS = L + (N_SEED + 1) + N_LEVELS * P_ORDER  # packed prologue tensor cols


def _device_mean_inputs(params: np.ndarray, bias: np.ndarray):
    """Small replicated tensors for the on-device companion-matrix scan.

    seed:  [64, 17] columns sigma_0..sigma_16
    ptN:   (M'^N)^T  for the three device doubling levels (lhsT layout)
    rhsa:  [65, 512] A'^T with the bias row appended
    """
    A, cb, Mp, dp = _derive_blocks(params, bias)
    rhsa = np.empty((P_ORDER + 1, L), np.float32)
    rhsa[:P_ORDER] = A.T.astype(np.float32)
    rhsa[P_ORDER] = cb.astype(np.float32)
    sig = np.zeros((N_SEED + 1, P_ORDER), np.float64)
    for j in range(N_SEED):
        sig[j + 1] = Mp @ sig[j] + dp
    smalls = np.zeros((P_ORDER + 1, SMALL_COLS), np.float32)
    smalls[:, :L] = rhsa
    smalls[:P_ORDER, L : L + N_SEED + 1] = sig.T.astype(np.float32)
    n = N_SEED
    c = L + N_SEED + 1
    Pn = np.linalg.matrix_power(Mp, N_SEED)
    while n * 2 <= NP_T:
        smalls[:P_ORDER, c : c + P_ORDER] = Pn.T.astype(np.float32)
        Pn = Pn @ Pn
        n *= 2
        c += P_ORDER
    return {"smalls": smalls}


_CACHE = {}


def _build_kernel():
    """Per-core program.

    Prologue (tiny, overlaps the noise streaming): companion-matrix doubling
    scan producing prefix states sigma_0..sigma_127 in Sa [65,128]
    (row 64 = ones for the bias term), then one [65,128]^T @ [65,512]
    matmul materializing means as a [128, 512] tile.

    Main: stream noise chunks, out = 0.3*noise + means (DVE scalar_tensor_tensor),
    loads on the sync HWDGE ring, stores on the scalar HWDGE ring.
    """
    P = P_ORDER
    nc = bacc.Bacc(None, target_bir_lowering=False)
    noise_d = nc.dram_tensor("noise", [ROWS, MAX_T], F32, kind="ExternalInput")
    smalls_d = nc.dram_tensor("smalls", [P + 1, SMALL_COLS], F32, kind="ExternalInput")
    out_d = nc.dram_tensor("out", [ROWS, MAX_T], F32, kind="ExternalOutput")

    add = mybir.AluOpType.add
    mult = mybir.AluOpType.mult

    with tile.TileContext(nc) as tc:
        with (
            tc.tile_pool(name="const", bufs=1) as cpool,
            tc.tile_pool(name="psum", bufs=2, space="PSUM") as pspool,
            tc.tile_pool(name="psum_m", bufs=1, space="PSUM") as psmpool,
            tc.tile_pool(name="work", bufs=1) as wpool,
        ):
            # ---- means prologue: doubling scan over blocks of L steps ----
            smalls = cpool.tile([P + 1, SMALL_COLS], F32)
            nc.sync.dma_start(out=smalls[:], in_=smalls_d[:])
            rhsa = smalls[:, 0:L]
            Sa = cpool.tile([P + 1, NP_T], F32)
            nc.vector.memset(Sa[P : P + 1, :], 1.0)  # ones row (bias term)
            nc.vector.tensor_copy(
                Sa[0:P, 0 : N_SEED + 1], smalls[0:P, L : L + N_SEED + 1]
            )

            n, c = N_SEED, L + N_SEED + 1
            while n * 2 <= NP_T:
                lo, hi = (n + 1, 2 * n + 1) if 2 * n < NP_T else (n + 1, 2 * n)
                w = hi - lo  # new columns sigma_{n+1}..
                ps = pspool.tile([P, NP_T // 2], F32, tag="ps")
                nc.tensor.matmul(ps[:, 0:w], smalls[0:P, c : c + P], Sa[0:P, 1 : 1 + w])
                nc.vector.tensor_scalar(
                    out=Sa[0:P, lo:hi],
                    in0=ps[:, 0:w],
                    scalar1=Sa[0:P, n : n + 1],
                    scalar2=None,
                    op0=add,
                )
                n, c = n * 2, c + P

            psm = psmpool.tile([NP_T, L], F32)
            nc.tensor.matmul(psm[:], Sa[:], rhsa)
            mtile = cpool.tile([NP_T, L], F32)
            nc.vector.tensor_copy(mtile[:], psm[:])
            mb = mtile[:].rearrange("p (o q) -> p o q", o=1).broadcast_to([NP_T, max(CHUNKS), L])

            # ---- memory-bound main loop ----
            r0 = 0
            for ch, g in enumerate(CHUNKS):
                t = wpool.tile([NP_T, g * L], F32, name=f"t{ch}", tag=f"t{ch}")
                src_ap = noise_d[r0 : r0 + g, :].rearrange("g (p q) -> p g q", p=NP_T)
                nc.sync.dma_start(
                    out=t[:].rearrange("p (g q) -> p g q", g=g), in_=src_ap
                )
                nc.vector.scalar_tensor_tensor(
                    out=t[:].rearrange("p (g q) -> p g q", g=g),
                    in0=t[:].rearrange("p (g q) -> p g q", g=g),
                    scalar=STD,
                    in1=mb[:, 0:g, :],
                    op0=mult,
                    op1=add,
                )
                dst = out_d[r0 : r0 + g, :].rearrange("g (p q) -> p g q", p=NP_T)
                nc.scalar.dma_start(out=dst, in_=t[:].rearrange("p (g q) -> p g q", g=g))
                r0 += g
    nc.finalize()
    return nc


def kernel(params: np.ndarray, bias: np.ndarray, noise: np.ndarray) -> np.ndarray:
    small = _device_mean_inputs(params, bias)
    if "nc" not in _CACHE:
        _CACHE["nc"] = _build_kernel()
    nc = _CACHE["nc"]
    in_maps = [
        {"noise": np.ascontiguousarray(noise[i * ROWS : (i + 1) * ROWS]), **small}
        for i in range(N_CORES)
    ]
    res = run_bass_kernel_spmd(nc, in_maps, core_ids=list(range(N_CORES)))
    return np.concatenate([r["out"] for r in res.results], axis=0)


# revision 24
# speedup vs baseline: 1.1251x; 1.1251x over previous
"""AR(64) trajectory sampler on 8 trn2 NeuronCores.

reference: means[t] = AR(64) recurrence (deterministic, shared across batch),
           out[b, t] = means[t] + 0.3 * noise[b, t],  noise [256, 65536] f32.

Strategy (per sharding hint): replicate params/bias-derived small tensors,
shard the noise batch dim across 8 cores (32 rows each). The length-T scan
is parallelized via the companion-matrix block formulation:
    means[512*p + q] = (sigma_p . A'[q]) + c'[q]*b ,  sigma_{p+1} = M' sigma_p + d'
so the device materializes means as one [65,128]^T @ [65,512] matmul and
then streams out = 0.3*noise + means (memory-bound part).

Host work is limited to deriving the O(p^2)-sized block matrices from the
64-element params vector (and, in HOST_MEANS mode, the 128-step state scan).
"""

import os
import sys

import numpy as np

for _p in ("/root/.axon_site/_ro/trn_rl_repo", "/opt/trn_rl_repo"):
    if _p not in sys.path and os.path.isdir(_p):
        sys.path.append(_p)

from concourse import bacc, bass, tile
from concourse import mybir
from concourse.bass_utils import run_bass_kernel_spmd

F32 = mybir.dt.float32

BATCH = 256
MAX_T = 65536
P_ORDER = 64
STD = 0.3
N_CORES = 8
ROWS = BATCH // N_CORES          # 32 noise rows per core
L = 512                          # block length; T partitions = MAX_T // L = 128
NP_T = MAX_T // L                # 128 partitions of the means tile
# chunk sizes in rows (512KB/row-pair): small first chunks let stores start
# early (mixed read+write sustains ~410GB/s vs ~385 single-direction); small
# last chunk shrinks the serial load->compute->store tail.
CHUNKS = [2, 2, 4, 4, 6, 6, 6, 2]
assert sum(CHUNKS) == ROWS


def _derive_blocks(params: np.ndarray, bias: np.ndarray):
    """Block-companion expansion of the AR(64) recurrence, in float64.

    Returns (A, cb, Mp, dp):
      A  [L, 64] : row q maps state sigma -> means offset q within a block
      cb [L]     : additive term (bias folded in)
      Mp [64,64] : state advance over one block of L steps
      dp [64]    : additive state term over one block
    with state sigma = [m_{t-1}, ..., m_{t-64}] (most-recent-first).
    """
    a = params.astype(np.float64)
    b = float(bias[0])
    p = P_ORDER
    U = np.zeros((L, p), np.float64)
    e = np.zeros(L, np.float64)
    for i in range(L):
        u = np.zeros(p, np.float64)
        if i < p:
            u[: p - i] += a[i:]
        kmax = min(i, p)
        if kmax:
            u += a[:kmax] @ U[i - kmax : i][::-1]
            e[i] = 1.0 + a[:kmax] @ e[i - kmax : i][::-1]
        else:
            e[i] = 1.0
        U[i] = u
    A = U
    cb = e * b
    Mp = A[L - p :][::-1].copy()
    dp = cb[L - p :][::-1].copy()
    return A, cb, Mp, dp


def _host_means_tile(params: np.ndarray, bias: np.ndarray) -> np.ndarray:
    """means as [128, 512] f32: row p = means[512p : 512(p+1)]."""
    A, cb, Mp, dp = _derive_blocks(params, bias)
    sig = np.zeros((NP_T, P_ORDER), np.float64)
    for j in range(NP_T - 1):
        sig[j + 1] = Mp @ sig[j] + dp
    means = sig @ A.T + cb[None, :]
    return means.astype(np.float32)


N_SEED = 16  # prefix states computed on host; device doubles 16 -> 128
N_LEVELS = 3  # 16 -> 32 -> 64 -> 128
SMALL_COLS = L + (N_SEED + 1) + N_LEVELS * P_ORDER  # packed prologue tensor cols


def _device_mean_inputs(params: np.ndarray, bias: np.ndarray):
    """Small replicated tensors for the on-device companion-matrix scan.

    seed:  [64, 17] columns sigma_0..sigma_16
    ptN:   (M'^N)^T  for the three device doubling levels (lhsT layout)
    rhsa:  [65, 512] A'^T with the bias row appended
    """
    A, cb, Mp, dp = _derive_blocks(params, bias)
    rhsa = np.empty((P_ORDER + 1, L), np.float32)
    rhsa[:P_ORDER] = A.T.astype(np.float32)
    rhsa[P_ORDER] = cb.astype(np.float32)
    sig = np.zeros((N_SEED + 1, P_ORDER), np.float64)
    for j in range(N_SEED):
        sig[j + 1] = Mp @ sig[j] + dp
    smalls = np.zeros((P_ORDER + 1, SMALL_COLS), np.float32)
    smalls[:, :L] = rhsa
    smalls[:P_ORDER, L : L + N_SEED + 1] = sig.T.astype(np.float32)
    n = N_SEED
    c = L + N_SEED + 1
    Pn = np.linalg.matrix_power(Mp, N_SEED)
    while n * 2 <= NP_T:
        smalls[:P_ORDER, c : c + P_ORDER] = Pn.T.astype(np.float32)
        Pn = Pn @ Pn
        n *= 2
        c += P_ORDER
    return {"smalls": smalls}


_CACHE = {}


def _build_kernel():
    """Per-core program.

    Prologue (tiny, overlaps the noise streaming): companion-matrix doubling
    scan producing prefix states sigma_0..sigma_127 in Sa [65,128]
    (row 64 = ones for the bias term), then one [65,128]^T @ [65,512]
    matmul materializing means as a [128, 512] tile.

    Main: stream noise chunks, out = 0.3*noise + means (DVE scalar_tensor_tensor),
    loads on the sync HWDGE ring, stores on the scalar HWDGE ring.
    """
    P = P_ORDER
    nc = bacc.Bacc(None, target_bir_lowering=False)
    noise_d = nc.dram_tensor("noise", [ROWS, MAX_T], F32, kind="ExternalInput")
    smalls_d = nc.dram_tensor("smalls", [P + 1, SMALL_COLS], F32, kind="ExternalInput")
    out_d = nc.dram_tensor("out", [ROWS, MAX_T], F32, kind="ExternalOutput")

    add = mybir.AluOpType.add
    mult = mybir.AluOpType.mult

    with tile.TileContext(nc) as tc:
        with (
            tc.tile_pool(name="const", bufs=1) as cpool,
            tc.tile_pool(name="psum", bufs=2, space="PSUM") as pspool,
            tc.tile_pool(name="psum_m", bufs=1, space="PSUM") as psmpool,
            tc.tile_pool(name="work", bufs=1) as wpool,
        ):
            # ---- means prologue: doubling scan over blocks of L steps ----
            smalls = cpool.tile([P + 1, SMALL_COLS], F32)
            nc.sync.dma_start(out=smalls[:], in_=smalls_d[:])
            rhsa = smalls[:, 0:L]
            Sa = cpool.tile([P + 1, NP_T], F32)
            nc.vector.memset(Sa[P : P + 1, :], 1.0)  # ones row (bias term)
            nc.vector.tensor_copy(
                Sa[0:P, 0 : N_SEED + 1], smalls[0:P, L : L + N_SEED + 1]
            )

            n, c = N_SEED, L + N_SEED + 1
            while n * 2 <= NP_T:
                lo, hi = (n + 1, 2 * n + 1) if 2 * n < NP_T else (n + 1, 2 * n)
                w = hi - lo  # new columns sigma_{n+1}..
                ps = pspool.tile([P, NP_T // 2], F32, tag="ps")
                nc.tensor.matmul(ps[:, 0:w], smalls[0:P, c : c + P], Sa[0:P, 1 : 1 + w])
                nc.vector.tensor_scalar(
                    out=Sa[0:P, lo:hi],
                    in0=ps[:, 0:w],
                    scalar1=Sa[0:P, n : n + 1],
                    scalar2=None,
                    op0=add,
                )
                n, c = n * 2, c + P

            psm = psmpool.tile([NP_T, L], F32)
            nc.tensor.matmul(psm[:], Sa[:], rhsa)
            mtile = cpool.tile([NP_T, L], F32)
            nc.vector.tensor_copy(mtile[:], psm[:])
            mb = mtile[:].rearrange("p (o q) -> p o q", o=1).broadcast_to([NP_T, max(CHUNKS), L])

            # ---- memory-bound main loop ----
            r0 = 0
            for ch, g in enumerate(CHUNKS):
                t = wpool.tile([NP_T, g * L], F32, name=f"t{ch}", tag=f"t{ch}")
                src_ap = noise_d[r0 : r0 + g, :].rearrange("g (p q) -> p g q", p=NP_T)
                nc.sync.dma_start(
                    out=t[:].rearrange("p (g q) -> p g q", g=g), in_=src_ap
                )
                nc.vector.scalar_tensor_tensor(
                    out=t[:].rearrange("p (g q) -> p g q", g=g),
                    in0=t[:].rearrange("p (g q) -> p g q", g=g),
                    scalar=STD,
                    in1=mb[:, 0:g, :],
                    op0=mult,
                    op1=add,
                )
                dst = out_d[r0 : r0 + g, :].rearrange("g (p q) -> p g q", p=NP_T)
                nc.scalar.dma_start(out=dst, in_=t[:].rearrange("p (g q) -> p g q", g=g))
                r0 += g
    nc.finalize()
    return nc


def kernel(params: np.ndarray, bias: np.ndarray, noise: np.ndarray) -> np.ndarray:
    small = _device_mean_inputs(params, bias)
    if "nc" not in _CACHE:
        _CACHE["nc"] = _build_kernel()
    nc = _CACHE["nc"]
    in_maps = [
        {"noise": np.ascontiguousarray(noise[i * ROWS : (i + 1) * ROWS]), **small}
        for i in range(N_CORES)
    ]
    res = run_bass_kernel_spmd(nc, in_maps, core_ids=list(range(N_CORES)))
    return np.concatenate([r["out"] for r in res.results], axis=0)
